# revision 1
# baseline (speedup 1.0000x reference)
"""TRN2 Bass kernel for ConvNeXt-MLP + parallel top-2-of-3 LoRA-MoE.

Data-parallel over the token dim across 8 NeuronCores (12544 tokens ->
1568/core). All weights replicated. Per core, everything is computed in
feature-major ("transposed") layout: activations live in SBUF as
[features_on_partitions, tokens_on_free_dim]; the host transposes x in and
the output back out.

Math per core (T = 1568 tokens):
  base:   outT = w2^T @ gelu(w1^T @ xT + b1) + b2          (f32r matmuls)
  router: logitsT = rw^T @ xT + rb (exact f32 on PE), transposed to
          token-major in 128-token chunks, softmax + top-2-of-3 +
          renormalize as dense per-expert weights, transposed back
  lora:   actT = gelu(wd^T @ xT); scaled = actT * expand(comb);
          moeT = wu^T @ scaled, accumulated into the same PSUM as the base

Hidden dim (3072 = 24 chunks) is processed in 4 groups of 6 chunks so that
w1/w2 stream through SBUF exactly once; the output accumulates per group in
6 PSUM banks and across groups in SBUF.
"""

import numpy as np

import concourse.bacc as bacc
import concourse.mybir as mybir
import concourse.tile as tile
from concourse.bass_utils import run_bass_kernel_spmd

F32 = mybir.dt.float32
F32R = mybir.dt.float32r
AF = mybir.ActivationFunctionType
ALU = mybir.AluOpType
AX = mybir.AxisListType

NCORES = 8
B, N, D = 64, 196, 768
T = B * N                  # 12544 tokens total
TC = T // NCORES           # 1568 tokens per core
HID = 4 * D                # 3072
E, R = 3, 8
ER = E * R                 # 24
DC = D // 128              # 6 input-feature chunks
HC = HID // 128            # 24 hidden chunks
MC = D // 128              # 6 output chunks
NGROUPS = 4
GH = HC // NGROUPS         # 6 hidden chunks per group
NT_SIZES = [392, 392, 392, 392]    # token tiles per core (sum = 1568)
RC_SIZES = [128] * 12 + [32]       # router token chunks (sum = 1568)

_cache = {}


def _build():
    nc = bacc.Bacc("TRN2", target_bir_lowering=False, debug=False)

    xt_d = nc.dram_tensor("xt", [D, TC], F32R, kind="ExternalInput")
    w1_d = nc.dram_tensor("w1", [D, HID], F32R, kind="ExternalInput")
    w2_d = nc.dram_tensor("w2", [HID, D], F32R, kind="ExternalInput")
    wu_d = nc.dram_tensor("wu", [ER, D], F32R, kind="ExternalInput")
    b1_d = nc.dram_tensor("b1r", [128, HC], F32, kind="ExternalInput")
    b2_d = nc.dram_tensor("b2r", [128, MC], F32, kind="ExternalInput")
    rwd_d = nc.dram_tensor("rwd", [D, 56], F32, kind="ExternalInput")
    rb_d = nc.dram_tensor("rb", [E, 1], F32, kind="ExternalInput")
    bx_d = nc.dram_tensor("bexp", [E, ER], F32R, kind="ExternalInput")
    id_d = nc.dram_tensor("ident", [128, 128], F32, kind="ExternalInput")
    out_d = nc.dram_tensor("outT", [D, TC], F32, kind="ExternalOutput")

    with tile.TileContext(nc) as tc:
        with (
            tc.tile_pool(name="const", bufs=1) as cp,
            tc.tile_pool(name="big", bufs=1) as bp,
            tc.tile_pool(name="wts", bufs=2) as wp,
            tc.tile_pool(name="hbuf", bufs=3) as hp,
            tc.tile_pool(name="small", bufs=2) as sp,
        ):
            # ---- resident loads ----
            # small constants go on the gpsimd DMA queue so they don't sit
            # behind the 4.8MB xt stream on the sync queue
            rwd = cp.tile([128, DC * 56], F32, tag="rwd")
            nc.gpsimd.dma_start(
                rwd[:].rearrange("p (c e) -> p c e", c=DC),
                rwd_d.rearrange("(c p) e -> p c e", p=128),
            )
            wu = cp.tile([ER, D], F32R, tag="wu")
            nc.gpsimd.dma_start(wu[:], wu_d[:])
            b1 = cp.tile([128, HC], F32, tag="b1")
            nc.gpsimd.dma_start(b1[:], b1_d[:])
            b2 = cp.tile([128, MC], F32, tag="b2")
            nc.gpsimd.dma_start(b2[:], b2_d[:])
            rb = cp.tile([E, 1], F32, tag="rb")
            nc.gpsimd.dma_start(rb[:], rb_d[:])
            bx = cp.tile([E, ER], F32R, tag="bx")
            nc.gpsimd.dma_start(bx[:], bx_d[:])
            ident = cp.tile([128, 128], F32, tag="ident")
            nc.gpsimd.dma_start(ident[:], id_d[:])

            # xt as one tile per token tile so deps are per token range
            xts = []
            t0 = 0
            for i, n in enumerate(NT_SIZES):
                x_i = bp.tile([128, DC * n], F32R, tag=f"xt{i}",
                              name=f"xt{i}")
                h = n // 2
                for lo, hi in ((0, h), (h, n)):
                    nc.sync.dma_start(
                        x_i[:].rearrange("p (c t) -> p c t", c=DC)[:, :, lo:hi],
                        xt_d.rearrange("(c p) t -> p c t", p=128)
                            [:, :, t0 + lo:t0 + hi],
                    )
                xts.append(x_i)
                t0 += n

            lgT = bp.tile([E, TC], F32, tag="lgT")
            comb_t = bp.tile([E, TC], F32R, tag="combt")
            scaled = bp.tile([ER, TC], F32R, tag="scaled")
            acc = bp.tile([128, MC * TC], F32, tag="acc")

            # ---- phase A: router (exact f32) + LoRA expert activations ----
            # PE order: logits MMs -> logit transposes -> LoRA down MMs ->
            # comb transposes -> expand MMs. The DVE softmax chains overlap
            # the LoRA matmuls so the PE never waits on them.
            lgtok = bp.tile([128, 3 * len(RC_SIZES)], F32, tag="lgtok")
            combtok = bp.tile([128, 3 * len(RC_SIZES)], F32, tag="combtok")
            acts = bp.tile([ER, TC], F32, tag="acts")
            with tc.tile_pool(name="psA", bufs=1, space="PSUM") as psA:
                # merged router logits + LoRA down, exact f32:
                # [3+24, n] = [rw | wd]^T @ xT
                t0 = 0
                for i, n in enumerate(NT_SIZES):
                    dn27 = psA.tile([56, 512], F32, tag="dn27", bufs=2,
                                    name=f"dn27_{t0}")
                    for c in range(DC):
                        nc.tensor.matmul(
                            dn27[:, :n],
                            rwd[:, c * 56:(c + 1) * 56],
                            xts[i][:, c * n:(c + 1) * n].bitcast(F32),
                            start=(c == 0), stop=(c == DC - 1),
                        )
                    nc.vector.tensor_scalar_add(lgT[:, t0:t0 + n],
                                                dn27[:E, :n], rb[:])
                    nc.scalar.activation(acts[:, t0:t0 + n], dn27[32:, :n],
                                         AF.Gelu)
                    t0 += n

                # transpose logits to token-major, 128-token chunks
                t0 = 0
                for ci, n in enumerate(RC_SIZES):
                    lg = psA.tile([128, E], F32, tag="lg", bufs=2,
                                  name=f"lg_{t0}")
                    nc.tensor.transpose(lg[:n, :], lgT[:, t0:t0 + n],
                                        ident[:E, :E])
                    nc.vector.tensor_copy(lgtok[:n, 3 * ci:3 * ci + 3],
                                          lg[:n, :])
                    t0 += n

            # ---- phase B: base MLP + LoRA-up, hidden dim in 4 groups ----
            with (
                tc.tile_pool(name="psO", bufs=1, space="PSUM") as psO,
                tc.tile_pool(name="psH", bufs=2, space="PSUM") as psH,
            ):
                for g in range(NGROUPS):
                    w1g = wp.tile([128, DC * GH * 128], F32R, tag="w1g")
                    gw = GH * 128
                    for lo, hi in ((0, gw // 2), (gw // 2, gw)):
                        nc.sync.dma_start(
                            w1g[:].rearrange("p (c f) -> p c f", c=DC)
                               [:, :, lo:hi],
                            w1_d.rearrange("(c p) f -> p c f", p=128)
                                [:, :, g * gw + lo:g * gw + hi],
                        )
                    w2g = wp.tile([128, GH * D], F32R, tag="w2g")
                    for lo, hi in ((0, GH // 2), (GH // 2, GH)):
                        nc.sync.dma_start(
                            w2g[:].rearrange("p (c f) -> p c f", c=GH)
                               [:, lo:hi, :],
                            w2_d.rearrange("(c p) f -> p c f", p=128)
                                [:, g * GH + lo:g * GH + hi, :],
                        )
                    t0 = 0
                    for nt, n in enumerate(NT_SIZES):
                        outp = [psO.tile([128, 512], F32, tag=f"out{m}",
                                         name=f"out{m}_{g}_{nt}")
                                for m in range(MC)]
                        hsb = [None] * GH
                        for j in range(GH + 1):
                            if j < GH:
                                hps = psH.tile([128, 512], F32, tag="h",
                                               name=f"h_{g}_{nt}_{j}")
                                for c in range(DC):
                                    nc.tensor.matmul(
                                        hps[:, :n],
                                        w1g[:, (c * GH + j) * 128:
                                               (c * GH + j) * 128 + 128],
                                        xts[nt][:, c * n:(c + 1) * n],
                                        start=(c == 0), stop=(c == DC - 1),
                                    )
                                hsb[j] = hp.tile([128, 512], F32R, tag="hs",
                                                 name=f"hs_{g}_{nt}_{j}")
                                nc.scalar.activation(
                                    hsb[j][:, :n], hps[:, :n], AF.Gelu,
                                    bias=b1[:, g * GH + j:g * GH + j + 1],
                                )
                            if j >= 1:
                                jj = j - 1
                                for m in range(MC):
                                    nc.tensor.matmul(
                                        outp[m][:, :n],
                                        w2g[:, jj * D + m * 128:
                                               jj * D + m * 128 + 128],
                                        hsb[jj][:, :n],
                                        start=(jj == 0),
                                        stop=(jj == GH - 1 and g < NGROUPS - 1),
                                    )
                        if g == NGROUPS - 1:
                            for m in range(MC):
                                nc.tensor.matmul(
                                    outp[m][:, :n],
                                    wu[:, m * 128:(m + 1) * 128],
                                    scaled[:, t0:t0 + n],
                                    start=False, stop=True,
                                )
                        for m in range(MC):
                            a = acc[:, m * TC + t0:m * TC + t0 + n]
                            if g == 0:
                                nc.vector.tensor_copy(a, outp[m][:, :n])
                            elif g < NGROUPS - 1:
                                nc.vector.tensor_add(a, a, outp[m][:, :n])
                            else:
                                nc.vector.scalar_tensor_tensor(
                                    a, outp[m][:, :n], b2[:, m:m + 1], a,
                                    op0=ALU.add, op1=ALU.add,
                                )
                        if g == NGROUPS - 1:
                            # one consolidated store for all 6 output chunks
                            nc.sync.dma_start(
                                out_d.rearrange("(m p) t -> p m t", p=128)
                                     [:, :, t0:t0 + n],
                                acc[:].rearrange("p (m t) -> p m t", m=MC)
                                      [:, :, t0:t0 + n],
                            )
                        t0 += n

                    if g == 0:
                        # softmax + top-2 combine weights; DVE is free now,
                        # all probs from one table-stable Exp (logits are
                        # O(1), no max-subtraction needed in fp32)
                        probs = bp.tile([128, 3 * len(RC_SIZES)], F32,
                                        tag="probs")
                        nc.scalar.activation(probs[:], lgtok[:], AF.Exp)
                        tq = 0
                        for ci, n in enumerate(RC_SIZES):
                            pr = probs[:, 3 * ci:3 * ci + 3]
                            ssum = sp.tile([128, 1], F32, tag="ssum",
                                           name=f"ssum_{tq}")
                            nc.vector.tensor_reduce(ssum[:n], pr[:n, :],
                                                    axis=AX.X, op=ALU.add)
                            pmin = sp.tile([128, 1], F32, tag="pmin",
                                           name=f"pmin_{tq}")
                            nc.vector.tensor_reduce(pmin[:n], pr[:n, :],
                                                    axis=AX.X, op=ALU.min)
                            rs = sp.tile([128, 1], F32, tag="rs",
                                         name=f"rs_{tq}")
                            nc.vector.reciprocal(rs[:n], ssum[:n])
                            d0 = sp.tile([128, 1], F32, tag="d0",
                                         name=f"d0_{tq}")
                            nc.vector.tensor_sub(d0[:n], ssum[:n], pmin[:n])
                            den = sp.tile([128, 1], F32, tag="den",
                                          name=f"den_{tq}")
                            nc.vector.tensor_scalar(den[:n], d0[:n], rs[:n],
                                                    1e-6, op0=ALU.mult,
                                                    op1=ALU.add)
                            invd = sp.tile([128, 1], F32, tag="invd",
                                           name=f"invd_{tq}")
                            nc.vector.reciprocal(invd[:n], den[:n])
                            t1 = sp.tile([128, 1], F32, tag="t1",
                                         name=f"t1_{tq}")
                            nc.vector.tensor_mul(t1[:n], rs[:n], invd[:n])
                            mask = sp.tile([128, E], F32, tag="mask",
                                           name=f"mask_{tq}")
                            nc.vector.tensor_scalar(mask[:n, :], pr[:n, :],
                                                    pmin[:n], None,
                                                    op0=ALU.is_gt)
                            nc.vector.scalar_tensor_tensor(
                                combtok[:n, 3 * ci:3 * ci + 3], mask[:n, :],
                                t1[:n], pr[:n, :], op0=ALU.mult, op1=ALU.mult,
                            )
                            tq += n

                    if g == 1:
                        # comb transposes + expand, overlapped behind group 1
                        tq = 0
                        for ci, n in enumerate(RC_SIZES):
                            tp = psH.tile([E, 128], F32, tag="h",
                                          name=f"tp_{tq}")
                            nc.tensor.transpose(tp[:, :n],
                                                combtok[:n, 3 * ci:3 * ci + 3],
                                                ident[:n, :n])
                            nc.scalar.copy(comb_t[:, tq:tq + n], tp[:, :n])
                            tq += n
                        tq = 0
                        for i, n in enumerate(NT_SIZES):
                            ex = psH.tile([ER, 512], F32, tag="h",
                                          name=f"ex_{tq}")
                            nc.tensor.matmul(ex[:, :n], bx[:],
                                             comb_t[:, tq:tq + n],
                                             start=True, stop=True)
                            nc.vector.tensor_mul(scaled[:, tq:tq + n],
                                                 acts[:, tq:tq + n], ex[:, :n])
                            tq += n

    nc.compile()
    return nc


def _pack_rwd(router_w, w_down):
    rwd = np.zeros((D, 56), np.float32)
    rwd[:, :E] = np.asarray(router_w, np.float32)
    rwd[:, 32:] = np.asarray(w_down, np.float32).transpose(1, 0, 2).reshape(D, ER)
    return rwd


def _prep_inputs(x, w1, b1, w2, b2, router_w, router_b, w_down, w_up):
    x = np.ascontiguousarray(np.asarray(x, dtype=np.float32))
    xT = x.reshape(T, D).T  # [D, T]
    common = {
        "w1": np.ascontiguousarray(np.asarray(w1, np.float32)),
        "w2": np.ascontiguousarray(np.asarray(w2, np.float32)),
        "wu": np.ascontiguousarray(np.asarray(w_up, np.float32).reshape(ER, D)),
        "b1r": np.ascontiguousarray(
            np.asarray(b1, np.float32).reshape(HC, 128).T),
        "b2r": np.ascontiguousarray(
            np.asarray(b2, np.float32).reshape(MC, 128).T),
        "rwd": _pack_rwd(router_w, w_down),
        "rb": np.ascontiguousarray(
            np.asarray(router_b, np.float32).reshape(E, 1)),
        "bexp": np.repeat(np.eye(E, dtype=np.float32), R, axis=1),
        "ident": np.eye(128, dtype=np.float32),
    }
    in_maps = []
    for c in range(NCORES):
        m = dict(common)
        m["xt"] = np.ascontiguousarray(xT[:, c * TC:(c + 1) * TC])
        in_maps.append(m)
    return in_maps


def _run(inputs, trace=False):
    if "nc" not in _cache:
        _cache["nc"] = _build()
    nc = _cache["nc"]
    in_maps = _prep_inputs(**inputs)
    res = run_bass_kernel_spmd(nc, in_maps, core_ids=list(range(NCORES)),
                               trace=trace)
    outT = np.concatenate([res.results[c]["outT"] for c in range(NCORES)],
                          axis=1)  # [D, T]
    out = np.ascontiguousarray(outT.T).reshape(B, N, D).astype(np.float32)
    return out, res


def kernel(**inputs):
    return _run(inputs)[0]



# revision 3
# speedup vs baseline: 1.0193x; 1.0193x over previous
"""TRN2 Bass kernel for ConvNeXt-MLP + parallel top-2-of-3 LoRA-MoE.

Data-parallel over the token dim across 8 NeuronCores (12544 tokens ->
1568/core). All weights replicated. Per core, everything is computed in
feature-major ("transposed") layout: activations live in SBUF as
[features_on_partitions, tokens_on_free_dim]; the host transposes x in and
the output back out.

Per core (T = 1568 tokens, 4 token tiles of 392):
  base:   outT = w2^T @ gelu(w1^T @ xT + b1) + b2          (f32r matmuls)
  router: merged [rw|wd]^T @ xT in f32r (1 cy/row); logits go token-major
          via DVE stream-transposes (32x32 blocks) + partition-shifted
          copies -- no PE small-ops, so the PE p-state never dips.
          Softmax + top-2-of-3 + renormalize run as ~11 batched DVE ops
          over all 13 token chunks at once (stride-0 broadcast APs);
          router bias is folded in as exp(rb) since softmax is shift/scale
          invariant: p_e ~ exp(l_e)*exp(b_e).
  lora:   actT = gelu(wd^T @ xT); scaled = actT * expand(comb);
          moeT = wu^T @ scaled, accumulated into the same PSUM as the base

Hidden dim (3072 = 24 chunks) is processed in 4 groups of 6 chunks so that
w1/w2 stream through SBUF exactly once; the output accumulates per group in
6 PSUM banks and across groups in SBUF. The router matmul for token tile i
is emitted right before tile i's group-0 matmuls (sharing the psH PSUM tag)
so the PE follows the xt DMA stream with no stalls. DMA queues: sync = xt +
output stores, vector = w1 groups, gpsimd = consts + w2 groups.
"""

import numpy as np

import concourse.bacc as bacc
import concourse.mybir as mybir
import concourse.tile as tile
from concourse.bass_utils import run_bass_kernel_spmd

F32 = mybir.dt.float32
F32R = mybir.dt.float32r
AF = mybir.ActivationFunctionType
ALU = mybir.AluOpType
AX = mybir.AxisListType

NCORES = 8
B, N, D = 64, 196, 768
T = B * N                  # 12544 tokens total
TC = T // NCORES           # 1568 tokens per core
HID = 4 * D                # 3072
E, R = 3, 8
ER = E * R                 # 24
DC = D // 128              # 6 input-feature chunks
HC = HID // 128            # 24 hidden chunks
MC = D // 128              # 6 output chunks
NGROUPS = 4
GH = HC // NGROUPS         # 6 hidden chunks per group
NT_SIZES = [392, 392, 392, 392]    # token tiles per core (sum = 1568)
NRC = 13                           # router 128-token chunks (12x128 + 32)
RC_N = [128] * 12 + [32]
# chunks fully covered once tile i's logits are written (tile bound 392*(i+1))
CHUNKS_BY_TILE = [[0, 1, 2], [3, 4, 5], [6, 7, 8], [9, 10, 11, 12]]

_cache = {}


def _build():
    nc = bacc.Bacc("TRN2", target_bir_lowering=False, debug=False)

    xt_d = nc.dram_tensor("xt", [D, TC], F32R, kind="ExternalInput")
    w1_d = nc.dram_tensor("w1", [D, HID], F32R, kind="ExternalInput")
    w2_d = nc.dram_tensor("w2", [HID, D], F32R, kind="ExternalInput")
    wu_d = nc.dram_tensor("wu", [ER, D], F32R, kind="ExternalInput")
    b1_d = nc.dram_tensor("b1r", [128, HC], F32, kind="ExternalInput")
    b2_d = nc.dram_tensor("b2r", [128, MC], F32, kind="ExternalInput")
    rwd_d = nc.dram_tensor("rwd", [D, 56], F32R, kind="ExternalInput")
    erb_d = nc.dram_tensor("erb", [128, E], F32, kind="ExternalInput")
    bx_d = nc.dram_tensor("bexp", [E, ER], F32R, kind="ExternalInput")
    out_d = nc.dram_tensor("outT", [D, TC], F32, kind="ExternalOutput")

    with tile.TileContext(nc) as tc:
        with (
            tc.tile_pool(name="const", bufs=1) as cp,
            tc.tile_pool(name="big", bufs=1) as bp,
            tc.tile_pool(name="wts", bufs=2) as wp,
            tc.tile_pool(name="hbuf", bufs=3) as hp,
        ):
            # ---- resident loads ----
            # small constants on the gpsimd queue, ahead of the w2 stream
            rwd = cp.tile([128, DC * 56], F32R, tag="rwd")
            nc.gpsimd.dma_start(
                rwd[:].rearrange("p (c e) -> p c e", c=DC),
                rwd_d.rearrange("(c p) e -> p c e", p=128),
            )
            wu = cp.tile([ER, D], F32R, tag="wu")
            nc.gpsimd.dma_start(wu[:], wu_d[:])
            b1 = cp.tile([128, HC], F32, tag="b1")
            nc.gpsimd.dma_start(b1[:], b1_d[:])
            b2 = cp.tile([128, MC], F32, tag="b2")
            nc.gpsimd.dma_start(b2[:], b2_d[:])
            erb = cp.tile([128, E], F32, tag="erb")
            nc.gpsimd.dma_start(erb[:], erb_d[:])
            bx = cp.tile([E, ER], F32R, tag="bx")
            nc.gpsimd.dma_start(bx[:], bx_d[:])

            # xt on the sync queue, one tile per token tile, split by
            # feature-chunk halves so phase-A matmul c=0..2 can start early
            xts = []
            t0 = 0
            for i, n in enumerate(NT_SIZES):
                x_i = bp.tile([128, DC * n], F32R, tag=f"xt{i}",
                              name=f"xt{i}")
                for lo, hi in ((0, DC // 2), (DC // 2, DC)):
                    nc.sync.dma_start(
                        x_i[:].rearrange("p (c t) -> p c t", c=DC)[:, lo:hi],
                        xt_d.rearrange("(c p) t -> p c t", p=128)
                            [:, lo:hi, t0:t0 + n],
                    )
                xts.append(x_i)
                t0 += n

            lgT = bp.tile([E, TC], F32, tag="lgT")
            acts = bp.tile([ER, TC], F32, tag="acts")
            comb_t = bp.tile([E, TC], F32R, tag="combt")
            scaled = bp.tile([ER, TC], F32R, tag="scaled")
            acc = bp.tile([128, MC * TC], F32, tag="acc")
            # token-major staging: per chunk ci a [128, 32] block; logits /
            # comb live in cols 32*ci .. 32*ci+2
            stg = bp.tile([128, NRC * 32], F32, tag="stg")
            ttok = bp.tile([128, NRC * 32], F32, tag="ttok")
            ctok = bp.tile([128, NRC * 32], F32, tag="ctok")
            ctokT = bp.tile([128, NRC * 32], F32, tag="ctokT")
            prb = bp.tile([128, NRC * 3], F32, tag="prb")
            ssum = bp.tile([128, NRC], F32, tag="ssum")
            pmin = bp.tile([128, NRC], F32, tag="pmin")
            rs = bp.tile([128, NRC], F32, tag="rs")
            d0 = bp.tile([128, NRC], F32, tag="d0")
            den = bp.tile([128, NRC], F32, tag="den")
            invd = bp.tile([128, NRC], F32, tag="invd")
            t1 = bp.tile([128, NRC], F32, tag="t1")
            msk = bp.tile([128, NRC * 3], F32, tag="msk")
            mp = bp.tile([128, NRC * 3], F32, tag="mp")

            def stage_logit_chunks(cis):
                """lgT [3,TC] -> token-major ttok staging, on DVE only."""
                for ci in cis:
                    nblk = RC_N[ci] // 32
                    for k in range(nblk):
                        lo = 128 * ci + 32 * k
                        nc.vector.tensor_copy(
                            stg[32 * k:32 * k + 3, 32 * ci:32 * ci + 32],
                            lgT[0:3, lo:lo + 32],
                        )
                    nc.vector.transpose(
                        ttok[:32 * nblk, 32 * ci:32 * ci + 32],
                        stg[:32 * nblk, 32 * ci:32 * ci + 32],
                    )

            def softmax_comb():
                """Batched softmax + top-2-of-3 + renormalize over all
                chunks; writes comb into ctok staging."""
                t3 = ttok[:].rearrange("p (c x) -> p c x", c=NRC)[:, :, 0:3]
                p3 = prb[:].rearrange("p (c e) -> p c e", c=NRC)
                nc.scalar.activation(p3, t3, AF.Exp)
                e3 = erb[:].unsqueeze(1).broadcast_to([128, NRC, 3])
                nc.vector.tensor_tensor(p3, p3, e3, op=ALU.mult)
                nc.vector.tensor_reduce(ssum[:], p3, axis=AX.X, op=ALU.add)
                nc.vector.tensor_reduce(pmin[:], p3, axis=AX.X, op=ALU.min)
                nc.vector.reciprocal(rs[:], ssum[:])
                nc.vector.tensor_sub(d0[:], ssum[:], pmin[:])
                nc.vector.tensor_mul(den[:], d0[:], rs[:])
                nc.vector.tensor_scalar_add(den[:], den[:], 1e-6)
                nc.vector.reciprocal(invd[:], den[:])
                nc.vector.tensor_mul(t1[:], rs[:], invd[:])
                m3 = msk[:].rearrange("p (c e) -> p c e", c=NRC)
                pm3 = pmin[:].unsqueeze(2).broadcast_to([128, NRC, 3])
                nc.vector.tensor_tensor(m3, p3, pm3, op=ALU.is_gt)
                mp3 = mp[:].rearrange("p (c e) -> p c e", c=NRC)
                nc.vector.tensor_tensor(mp3, m3, p3, op=ALU.mult)
                c3 = ctok[:].rearrange("p (c x) -> p c x", c=NRC)[:, :, 0:3]
                t13 = t1[:].unsqueeze(2).broadcast_to([128, NRC, 3])
                nc.vector.tensor_tensor(c3, mp3, t13, op=ALU.mult)
                # comb back to expert-major [3, TC], still DVE-only
                for ci in range(NRC):
                    nblk = RC_N[ci] // 32
                    nc.vector.transpose(
                        ctokT[:32 * nblk, 32 * ci:32 * ci + 32],
                        ctok[:32 * nblk, 32 * ci:32 * ci + 32],
                    )
                    for k in range(nblk):
                        lo = 128 * ci + 32 * k
                        nc.vector.tensor_copy(
                            comb_t[0:3, lo:lo + 32],
                            ctokT[32 * k:32 * k + 3, 32 * ci:32 * ci + 32],
                        )

            def load_w1g(g):
                # w1 group on the scalar (Activation) DMA queue, split in
                # hidden-half chunks matching the j-loop read order
                w1g = wp.tile([128, DC * GH * 128], F32R, tag="w1g",
                              name=f"w1g_{g}")
                gw = GH * 128
                for lo, hi in ((0, gw // 2), (gw // 2, gw)):
                    nc.scalar.dma_start(
                        w1g[:].rearrange("p (c f) -> p c f", c=DC)
                           [:, :, lo:hi],
                        w1_d.rearrange("(c p) f -> p c f", p=128)
                            [:, :, g * gw + lo:g * gw + hi],
                    )
                return w1g

            def load_w2g(g):
                w2g = wp.tile([128, GH * D], F32R, tag="w2g",
                              name=f"w2g_{g}")
                for lo, hi in ((0, GH // 2), (GH // 2, GH)):
                    nc.gpsimd.dma_start(
                        w2g[:].rearrange("p (c f) -> p c f", c=GH)
                           [:, lo:hi, :],
                        w2_d.rearrange("(c p) f -> p c f", p=128)
                            [:, g * GH + lo:g * GH + hi, :],
                    )
                return w2g

            # ---- main stream: 4 groups of 6 hidden chunks ----
            with (
                tc.tile_pool(name="psO", bufs=1, space="PSUM") as psO,
                tc.tile_pool(name="psH", bufs=2, space="PSUM") as psH,
            ):
                # prefetch one group ahead so the issue ops never stall the
                # gelu stream and transfers land before they're needed
                w1q = [load_w1g(0), load_w1g(1)]
                w2q = [load_w2g(0), load_w2g(1)]
                for g in range(NGROUPS):
                    if g >= 1 and g + 1 < NGROUPS:
                        w1q.append(load_w1g(g + 1))
                        w2q.append(load_w2g(g + 1))
                    w1g, w2g = w1q[g], w2q[g]

                    if g == 1:
                        # expand comb -> per-(expert,rank) scale, then apply
                        # to the LoRA activations. 4 dense 392-row matmuls.
                        tq = 0
                        for i, n in enumerate(NT_SIZES):
                            ex = psH.tile([128, 512], F32, tag="h",
                                          name=f"ex_{tq}")
                            nc.tensor.matmul(ex[:ER, :n], bx[:],
                                             comb_t[:, tq:tq + n],
                                             start=True, stop=True)
                            nc.vector.tensor_mul(scaled[:, tq:tq + n],
                                                 acts[:, tq:tq + n],
                                                 ex[:ER, :n])
                            tq += n

                    t0 = 0
                    for nt, n in enumerate(NT_SIZES):
                        if g == 0:
                            # merged router + LoRA-down matmul for this tile
                            dn27 = psH.tile([128, 512], F32, tag="h",
                                            name=f"dn27_{nt}")
                            for c in range(DC):
                                nc.tensor.matmul(
                                    dn27[:56, :n],
                                    rwd[:, c * 56:(c + 1) * 56],
                                    xts[nt][:, c * n:(c + 1) * n],
                                    start=(c == 0), stop=(c == DC - 1),
                                )
                            nc.vector.tensor_copy(lgT[:, t0:t0 + n],
                                                  dn27[:E, :n])
                            nc.scalar.activation(acts[:, t0:t0 + n],
                                                 dn27[32:56, :n], AF.Gelu)
                            stage_logit_chunks(CHUNKS_BY_TILE[nt])
                            if nt == len(NT_SIZES) - 1:
                                softmax_comb()

                        outp = [psO.tile([128, 512], F32, tag=f"out{m}",
                                         name=f"out{m}_{g}_{nt}")
                                for m in range(MC)]
                        hsb = [None] * GH
                        for j in range(GH + 1):
                            if j < GH:
                                hps = psH.tile([128, 512], F32, tag="h",
                                               name=f"h_{g}_{nt}_{j}")
                                for c in range(DC):
                                    nc.tensor.matmul(
                                        hps[:, :n],
                                        w1g[:, (c * GH + j) * 128:
                                               (c * GH + j) * 128 + 128],
                                        xts[nt][:, c * n:(c + 1) * n],
                                        start=(c == 0), stop=(c == DC - 1),
                                    )
                                hsb[j] = hp.tile([128, 512], F32R, tag="hs",
                                                 name=f"hs_{g}_{nt}_{j}")
                                nc.scalar.activation(
                                    hsb[j][:, :n], hps[:, :n], AF.Gelu,
                                    bias=b1[:, g * GH + j:g * GH + j + 1],
                                )
                            if j >= 1:
                                jj = j - 1
                                for m in range(MC):
                                    nc.tensor.matmul(
                                        outp[m][:, :n],
                                        w2g[:, jj * D + m * 128:
                                               jj * D + m * 128 + 128],
                                        hsb[jj][:, :n],
                                        start=(jj == 0),
                                        stop=(jj == GH - 1 and g < NGROUPS - 1),
                                    )
                        if g == NGROUPS - 1:
                            for m in range(MC):
                                nc.tensor.matmul(
                                    outp[m][:, :n],
                                    wu[:, m * 128:(m + 1) * 128],
                                    scaled[:, t0:t0 + n],
                                    start=False, stop=True,
                                )
                        for m in range(MC):
                            a = acc[:, m * TC + t0:m * TC + t0 + n]
                            if g == 0:
                                nc.vector.tensor_copy(a, outp[m][:, :n])
                            elif g < NGROUPS - 1:
                                nc.vector.tensor_add(a, a, outp[m][:, :n])
                            else:
                                nc.vector.scalar_tensor_tensor(
                                    a, outp[m][:, :n], b2[:, m:m + 1], a,
                                    op0=ALU.add, op1=ALU.add,
                                )
                                # store each output chunk as soon as its
                                # bias-add lands (short tail)
                                nc.sync.dma_start(
                                    out_d.rearrange("(m p) t -> p m t", p=128)
                                         [:, m, t0:t0 + n],
                                    a,
                                )
                        t0 += n

    nc.compile()
    return nc


def _pack_rwd(router_w, w_down):
    rwd = np.zeros((D, 56), np.float32)
    rwd[:, :E] = np.asarray(router_w, np.float32)
    rwd[:, 32:] = np.asarray(w_down, np.float32).transpose(1, 0, 2).reshape(D, ER)
    return rwd


def _prep_inputs(x, w1, b1, w2, b2, router_w, router_b, w_down, w_up):
    x = np.ascontiguousarray(np.asarray(x, dtype=np.float32))
    xT = x.reshape(T, D).T  # [D, T]
    common = {
        "w1": np.ascontiguousarray(np.asarray(w1, np.float32)),
        "w2": np.ascontiguousarray(np.asarray(w2, np.float32)),
        "wu": np.ascontiguousarray(np.asarray(w_up, np.float32).reshape(ER, D)),
        "b1r": np.ascontiguousarray(
            np.asarray(b1, np.float32).reshape(HC, 128).T),
        "b2r": np.ascontiguousarray(
            np.asarray(b2, np.float32).reshape(MC, 128).T),
        "rwd": _pack_rwd(router_w, w_down),
        "erb": np.ascontiguousarray(np.broadcast_to(
            np.exp(np.asarray(router_b, np.float32))[None, :], (128, E))),
        "bexp": np.repeat(np.eye(E, dtype=np.float32), R, axis=1),
    }
    in_maps = []
    for c in range(NCORES):
        m = dict(common)
        m["xt"] = np.ascontiguousarray(xT[:, c * TC:(c + 1) * TC])
        in_maps.append(m)
    return in_maps


def _run(inputs, trace=False):
    if "nc" not in _cache:
        _cache["nc"] = _build()
    nc = _cache["nc"]
    in_maps = _prep_inputs(**inputs)
    res = run_bass_kernel_spmd(nc, in_maps, core_ids=list(range(NCORES)),
                               trace=trace)
    outT = np.concatenate([res.results[c]["outT"] for c in range(NCORES)],
                          axis=1)  # [D, T]
    out = np.ascontiguousarray(outT.T).reshape(B, N, D).astype(np.float32)
    return out, res


def kernel(**inputs):
    return _run(inputs)[0]


# revision 6
# speedup vs baseline: 1.0379x; 1.0182x over previous
"""TRN2 Bass kernel for ConvNeXt-MLP + parallel top-2-of-3 LoRA-MoE.

Data-parallel over the token dim across 8 NeuronCores (12544 tokens ->
1568/core). All weights replicated. Per core, everything runs in
feature-major ("transposed") layout: activations live in SBUF as
[features_on_partitions, tokens_on_free_dim]; the host transposes x in and
the output back out, and pre-tiles x/w1/w2/consts into the exact SBUF
layouts so every DMA is a contiguous block.

Per core (T = 1568 tokens, 4 token tiles of 392):
  base:   outT = w2^T @ gelu(w1^T @ xT + b1) + b2          (f32r matmuls)
  router: merged [rw|wd]^T @ xT in f32r (1 cy/row). Logits go token-major
          via DVE stream-transposes (32x32 blocks) + partition-shifted
          copies, overlapped tile-by-tile under the main matmul stream.
          Softmax + top-2-of-3 + renormalize run as ~11 batched DVE ops
          over all 13 token chunks at once (stride-0 broadcast APs); the
          router bias is folded in as exp(rb) since softmax is shift/scale
          invariant. The Exp (which swaps the ACT table away from Gelu) is
          emitted at the g0->g1 boundary inside a w2-chain window where the
          ACT engine is idle. comb goes back to expert-major via 13 PE
          transposes interleaved one-at-a-time into g1's dense matmul
          stream (keeps the PE p-state hot).
  lora:   actT = gelu(wd^T @ xT); scaled = actT * expand(comb), expand
          matmuls interleaved into g2; moeT = wu^T @ scaled accumulated
          into the same PSUM banks as the base output in g3.

Hidden dim (3072 = 24 chunks) is processed in 4 groups of 6 chunks so that
w1/w2 stream through SBUF exactly once (prefetched one group ahead on the
scalar/gpsimd DMA queues; xt + output stores ride the sync queue). The
j-loop runs a depth-3 software pipeline (h j0..j2 before the first w2
chain) so the next tile's PSUM reuse never waits on this tile's accumulate
copies.
"""

import numpy as np

import concourse.bacc as bacc
import concourse.mybir as mybir
import concourse.tile as tile
from concourse.bass_utils import run_bass_kernel_spmd

F32 = mybir.dt.float32
F32R = mybir.dt.float32r
AF = mybir.ActivationFunctionType
ALU = mybir.AluOpType
AX = mybir.AxisListType

NCORES = 8
B, N, D = 64, 196, 768
T = B * N                  # 12544 tokens total
TC = T // NCORES           # 1568 tokens per core
HID = 4 * D                # 3072
E, R = 3, 8
ER = E * R                 # 24
DC = D // 128              # 6 input-feature chunks
HC = HID // 128            # 24 hidden chunks
MC = D // 128              # 6 output chunks
NGROUPS = 4
GH = HC // NGROUPS         # 6 hidden chunks per group
NT = 4
TN = TC // NT              # 392 tokens per tile
NRC = 13                   # router 128-token chunks (12x128 + 32)
RC_N = [128] * 12 + [32]
CHUNKS_BY_TILE = [[0, 1, 2], [3, 4, 5], [6, 7, 8], [9, 10, 11, 12]]

# const blob column offsets: f32r blob (PE-consumed) and f32 blob
RWD0, BX0, WU0, ID0 = 0, 336, 360, 1128
CWR = ID0 + 128
B10, B20, ERB0 = 0, 24, 30
CWF = 33

_cache = {}


def _build():
    nc = bacc.Bacc("TRN2", target_bir_lowering=False, debug=False)

    xt_d = nc.dram_tensor("xt", [NT * 128, DC * TN], F32R,
                          kind="ExternalInput")
    w1_d = nc.dram_tensor("w1", [NGROUPS * 128, GH * DC * 128], F32R,
                          kind="ExternalInput")
    w2_d = nc.dram_tensor("w2", [NGROUPS * 128, GH * D], F32R,
                          kind="ExternalInput")
    cbr_d = nc.dram_tensor("cblobr", [128, CWR], F32R, kind="ExternalInput")
    cbf_d = nc.dram_tensor("cblobf", [128, CWF], F32, kind="ExternalInput")
    out_d = nc.dram_tensor("outT", [NT * 128, MC * TN], F32,
                           kind="ExternalOutput")

    with tile.TileContext(nc) as tc:
        with (
            tc.tile_pool(name="const", bufs=1) as cp,
            tc.tile_pool(name="big", bufs=1) as bp,
            tc.tile_pool(name="wts", bufs=2) as wp,
            tc.tile_pool(name="hbuf", bufs=4) as hp,
        ):
            # ---- resident loads ----
            cbr = cp.tile([128, CWR], F32R, tag="cbr")
            nc.gpsimd.dma_start(cbr[:], cbr_d[:])
            cbf = cp.tile([128, CWF], F32, tag="cbf")
            nc.gpsimd.dma_start(cbf[:], cbf_d[:])
            rwd = cbr[:, RWD0:RWD0 + DC * 56]
            bx = cbr[0:E, BX0:BX0 + ER]
            wu = cbr[0:ER, WU0:WU0 + D]
            ident = cbr[:, ID0:ID0 + 128]
            b1 = cbf[:, B10:B10 + HC]
            b2 = cbf[:, B20:B20 + MC]
            erb = cbf[:, ERB0:ERB0 + E]

            xts = []
            for i in range(NT):
                x_i = bp.tile([128, DC * TN], F32R, tag=f"xt{i}",
                              name=f"xt{i}")
                half = DC * TN // 2
                for lo, hi in ((0, half), (half, 2 * half)):
                    nc.sync.dma_start(x_i[:, lo:hi],
                                      xt_d[i * 128:(i + 1) * 128, lo:hi])
                xts.append(x_i)

            lgT = bp.tile([E, TC], F32, tag="lgT")
            acts = bp.tile([ER, TC], F32, tag="acts")
            comb_t = bp.tile([E, TC], F32R, tag="combt")
            scaled = bp.tile([ER, TC], F32R, tag="scaled")
            acc = bp.tile([128, MC * TC], F32, tag="acc")
            stg = bp.tile([128, NRC * 32], F32, tag="stg")
            ttok = bp.tile([128, NRC * 32], F32, tag="ttok")
            ctok = bp.tile([128, NRC * 3], F32R, tag="ctok")
            prb = bp.tile([128, NRC * 3], F32, tag="prb")
            ssum = bp.tile([128, NRC], F32, tag="ssum")
            pmin = bp.tile([128, NRC], F32, tag="pmin")
            rs = bp.tile([128, NRC], F32, tag="rs")
            den = bp.tile([128, NRC], F32, tag="den")
            invd = bp.tile([128, NRC], F32, tag="invd")
            t1 = bp.tile([128, NRC], F32, tag="t1")
            msk = bp.tile([128, NRC * 3], F32, tag="msk")

            def stage_logit_chunks(cis):
                """lgT [3,TC] -> token-major ttok staging, on DVE only."""
                for ci in cis:
                    nblk = RC_N[ci] // 32
                    for k in range(nblk):
                        lo = 128 * ci + 32 * k
                        nc.vector.tensor_copy(
                            stg[32 * k:32 * k + 3, 32 * ci:32 * ci + 32],
                            lgT[0:3, lo:lo + 32],
                        )
                    nc.vector.transpose(
                        ttok[:32 * nblk, 32 * ci:32 * ci + 32],
                        stg[:32 * nblk, 32 * ci:32 * ci + 32],
                    )

            def softmax_comb():
                """Batched softmax + top-2-of-3 + renormalize over all
                chunks; writes token-major comb into ctok."""
                t3 = ttok[:].rearrange("p (c x) -> p c x", c=NRC)[:, :, 0:3]
                p3 = prb[:].rearrange("p (c e) -> p c e", c=NRC)
                nc.scalar.activation(p3, t3, AF.Exp)
                e3 = erb.unsqueeze(1).broadcast_to([128, NRC, 3])
                nc.vector.tensor_tensor(p3, p3, e3, op=ALU.mult)
                nc.vector.tensor_reduce(ssum[:], p3, axis=AX.X, op=ALU.add)
                nc.vector.tensor_reduce(pmin[:], p3, axis=AX.X, op=ALU.min)
                nc.vector.reciprocal(rs[:], ssum[:])
                nc.vector.tensor_sub(den[:], ssum[:], pmin[:])
                nc.vector.tensor_mul(den[:], den[:], rs[:])
                nc.vector.tensor_scalar_add(den[:], den[:], 1e-6)
                nc.vector.reciprocal(invd[:], den[:])
                nc.vector.tensor_mul(t1[:], rs[:], invd[:])
                m3 = msk[:].rearrange("p (c e) -> p c e", c=NRC)
                pm3 = pmin[:].unsqueeze(2).broadcast_to([128, NRC, 3])
                nc.vector.tensor_tensor(m3, p3, pm3, op=ALU.is_gt)
                nc.vector.tensor_tensor(m3, m3, p3, op=ALU.mult)
                c3 = ctok[:].rearrange("p (c e) -> p c e", c=NRC)
                t13 = t1[:].unsqueeze(2).broadcast_to([128, NRC, 3])
                nc.vector.tensor_tensor(c3, m3, t13, op=ALU.mult)

            def load_w1g(g):
                w1g = wp.tile([128, GH * DC * 128], F32R, tag="w1g",
                              name=f"w1g_{g}")
                half = GH * DC * 128 // 2
                for lo, hi in ((0, half), (half, 2 * half)):
                    nc.scalar.dma_start(
                        w1g[:, lo:hi], w1_d[g * 128:(g + 1) * 128, lo:hi])
                return w1g

            def load_w2g(g):
                w2g = wp.tile([128, GH * D], F32R, tag="w2g",
                              name=f"w2g_{g}")
                half = GH * D // 2
                for lo, hi in ((0, half), (half, 2 * half)):
                    nc.gpsimd.dma_start(
                        w2g[:, lo:hi], w2_d[g * 128:(g + 1) * 128, lo:hi])
                return w2g

            # deferred PE-side tasks, interleaved one per j-iteration into
            # the dense matmul stream so the PE array never cools down
            side_pe = []

            def emit_comb_transpose(ci, psH):
                n = RC_N[ci]
                tp = psH.tile([128, 512], F32, tag="h", name=f"tp_{ci}")
                nc.tensor.transpose(tp[:E, :n].bitcast(F32R),
                                    ctok[0:n, 3 * ci:3 * ci + 3],
                                    ident[:n, :n])
                nc.scalar.copy(comb_t[:, 128 * ci:128 * ci + n], tp[:E, :n])

            def emit_expand(i, psH):
                t0 = i * TN
                ex = psH.tile([128, 512], F32, tag="h", name=f"ex_{i}")
                nc.tensor.matmul(ex[:ER, :TN], bx, comb_t[:, t0:t0 + TN],
                                 start=True, stop=True)
                nc.vector.tensor_mul(scaled[:, t0:t0 + TN],
                                     acts[:, t0:t0 + TN], ex[:ER, :TN])

            # ---- main stream: 4 groups of 6 hidden chunks ----
            with (
                tc.tile_pool(name="psO", bufs=1, space="PSUM") as psO,
                tc.tile_pool(name="psH", bufs=2, space="PSUM") as psH,
            ):
                w1q = [load_w1g(0), load_w1g(1)]
                w2q = [load_w2g(0), load_w2g(1)]
                for g in range(NGROUPS):
                    if g >= 1 and g + 1 < NGROUPS:
                        w1q.append(load_w1g(g + 1))
                        w2q.append(load_w2g(g + 1))
                    w1g, w2g = w1q[g], w2q[g]

                    if g == 1:
                        # ACT is idle during g0-t3's w2 chains: do the Exp
                        # (and its two table swaps) there, then queue the 13
                        # comb transposes for interleaving into g1
                        softmax_comb()
                        side_pe.extend(
                            (lambda ci=ci: emit_comb_transpose(ci, psH))
                            for ci in range(NRC))
                    if g == 2:
                        side_pe.extend(
                            (lambda i=i: emit_expand(i, psH))
                            for i in range(NT))

                    t0 = 0
                    for nt in range(NT):
                        n = TN
                        if g == 0:
                            # merged router + LoRA-down matmul, this tile
                            dn27 = psH.tile([128, 512], F32, tag="h",
                                            name=f"dn27_{nt}")
                            for c in range(DC):
                                nc.tensor.matmul(
                                    dn27[:56, :n],
                                    rwd[:, c * 56:(c + 1) * 56],
                                    xts[nt][:, c * n:(c + 1) * n],
                                    start=(c == 0), stop=(c == DC - 1),
                                )
                            nc.vector.tensor_copy(lgT[:, t0:t0 + n],
                                                  dn27[:E, :n])
                            nc.scalar.activation(acts[:, t0:t0 + n],
                                                 dn27[32:56, :n], AF.Gelu)
                            stage_logit_chunks(CHUNKS_BY_TILE[nt])

                        outp = [psO.tile([128, 512], F32, tag=f"out{m}",
                                         name=f"out{m}_{g}_{nt}")
                                for m in range(MC)]
                        hsb = [None] * GH
                        for j in range(GH + 3):
                            if j < GH:
                                hps = psH.tile([128, 512], F32, tag="h",
                                               name=f"h_{g}_{nt}_{j}")
                                for c in range(DC):
                                    nc.tensor.matmul(
                                        hps[:, :n],
                                        w1g[:, (j * DC + c) * 128:
                                               (j * DC + c) * 128 + 128],
                                        xts[nt][:, c * n:(c + 1) * n],
                                        start=(c == 0), stop=(c == DC - 1),
                                    )
                                hsb[j] = hp.tile([128, 512], F32R, tag="hs",
                                                 name=f"hs_{g}_{nt}_{j}")
                                nc.scalar.activation(
                                    hsb[j][:, :n], hps[:, :n], AF.Gelu,
                                    bias=b1[:, g * GH + j:g * GH + j + 1],
                                )
                                if side_pe and j >= 2:
                                    side_pe.pop(0)()
                            if j >= 3:
                                jj = j - 3
                                for m in range(MC):
                                    nc.tensor.matmul(
                                        outp[m][:, :n],
                                        w2g[:, jj * D + m * 128:
                                               jj * D + m * 128 + 128],
                                        hsb[jj][:, :n],
                                        start=(jj == 0),
                                        stop=(jj == GH - 1 and g < NGROUPS - 1),
                                    )
                        if g == NGROUPS - 1:
                            for m in range(MC):
                                nc.tensor.matmul(
                                    outp[m][:, :n],
                                    wu[:, m * 128:(m + 1) * 128],
                                    scaled[:, t0:t0 + n],
                                    start=False, stop=True,
                                )
                        for m in range(MC):
                            a = acc[:, m * TC + t0:m * TC + t0 + n]
                            if g == 0:
                                nc.vector.tensor_copy(a, outp[m][:, :n])
                            elif g < NGROUPS - 1:
                                nc.vector.tensor_add(a, a, outp[m][:, :n])
                            else:
                                nc.vector.scalar_tensor_tensor(
                                    a, outp[m][:, :n], b2[:, m:m + 1], a,
                                    op0=ALU.add, op1=ALU.add,
                                )
                                nc.sync.dma_start(
                                    out_d[nt * 128:(nt + 1) * 128,
                                          m * n:(m + 1) * n],
                                    a,
                                )
                        t0 += n

    nc.compile()
    return nc


def _pack_consts(b1, b2, router_w, router_b, w_down, w_up):
    cbr = np.zeros((128, CWR), np.float32)
    rwd = np.zeros((DC, 128, 56), np.float32)
    rw = np.asarray(router_w, np.float32).reshape(DC, 128, E)
    wd = np.asarray(w_down, np.float32).transpose(1, 0, 2).reshape(DC, 128, ER)
    rwd[:, :, :E] = rw
    rwd[:, :, 32:] = wd
    cbr[:, RWD0:RWD0 + DC * 56] = rwd.transpose(1, 0, 2).reshape(128, DC * 56)
    cbr[0:E, BX0:BX0 + ER] = np.repeat(np.eye(E, dtype=np.float32), R, axis=1)
    cbr[0:ER, WU0:WU0 + D] = np.asarray(w_up, np.float32).reshape(ER, D)
    cbr[:, ID0:ID0 + 128] = np.eye(128, dtype=np.float32)
    cbf = np.zeros((128, CWF), np.float32)
    cbf[:, B10:B10 + HC] = np.asarray(b1, np.float32).reshape(HC, 128).T
    cbf[:, B20:B20 + MC] = np.asarray(b2, np.float32).reshape(MC, 128).T
    cbf[:, ERB0:ERB0 + E] = np.exp(np.asarray(router_b, np.float32))[None, :]
    return cbr, cbf


def _prep_inputs(x, w1, b1, w2, b2, router_w, router_b, w_down, w_up):
    x = np.ascontiguousarray(np.asarray(x, dtype=np.float32))
    xT = x.reshape(T, D).T  # [D, T]
    # w1 [D, HID] -> [g, p, j, c, f128] -> [(g p), j*c*128]
    w1p = np.asarray(w1, np.float32).reshape(DC, 128, NGROUPS, GH, 128)
    w1p = np.ascontiguousarray(w1p.transpose(2, 1, 3, 0, 4)).reshape(
        NGROUPS * 128, GH * DC * 128)
    # w2 [HID, D] -> [g, p, j, dout] -> [(g p), j*D]
    w2p = np.asarray(w2, np.float32).reshape(NGROUPS, GH, 128, D)
    w2p = np.ascontiguousarray(w2p.transpose(0, 2, 1, 3)).reshape(
        NGROUPS * 128, GH * D)
    cbr, cbf = _pack_consts(b1, b2, router_w, router_b, w_down, w_up)
    common = {
        "w1": w1p,
        "w2": w2p,
        "cblobr": cbr,
        "cblobf": cbf,
    }
    in_maps = []
    for c in range(NCORES):
        m = dict(common)
        xc = xT[:, c * TC:(c + 1) * TC].reshape(DC, 128, NT, TN)
        m["xt"] = np.ascontiguousarray(xc.transpose(2, 1, 0, 3)).reshape(
            NT * 128, DC * TN)
        in_maps.append(m)
    return in_maps


def _run(inputs, trace=False):
    if "nc" not in _cache:
        _cache["nc"] = _build()
    nc = _cache["nc"]
    in_maps = _prep_inputs(**inputs)
    res = run_bass_kernel_spmd(nc, in_maps, core_ids=list(range(NCORES)),
                               trace=trace)
    outs = []
    for c in range(NCORES):
        a = res.results[c]["outT"].reshape(NT, 128, MC, TN)
        outs.append(a.transpose(2, 1, 0, 3).reshape(D, TC))
    outT = np.concatenate(outs, axis=1)  # [D, T]
    out = np.ascontiguousarray(outT.T).reshape(B, N, D).astype(np.float32)
    return out, res


def kernel(**inputs):
    return _run(inputs)[0]
